# revision 27
# baseline (speedup 1.0000x reference)
"""Multi-head cosine self-attention on 8 Trainium2 NeuronCores (Bass/Tile).

Problem: y = MHA(x) with L2-normalized q/k (cosine attention) and per-head
scaling sim / n**sigmoid(m);  x: [4, 2048, 1024], 16 heads of dim 64.

Sharding: core c handles batch c//2 and head-group c%2 (8 heads = 512 of the
1024 q/k/v features).  Each core computes its partial output
(attn_out_part @ Wo[rows]); the host sums the two partials per batch and adds
bo.  No collectives.

Per-core pipeline (bf16 datapath, fp32 PSUM accumulation):
  - xT blocks stream to SBUF bf16; v = x Wv + bv (x-tile stationary)
  - qT/kT = W.T @ xT, k-outer loop with 2-chunk PSUM accumulators so each
    stationary weight tile is reused across i-chunks
  - row norms: ones-block matmul -> [2, n] PSUM; 1/(||q||*n^sig) computed in
    ONE Abs_reciprocal_sqrt activation (scale=(n^sig)^2); broadcast to 128
    partitions via a K=2 indicator matmul and applied in-place (DVE)
  - all norms for all 4 head-pairs are emitted before attention, so the PE
    stream stays dense and the HAM clock stays at 2.4 GHz
  - simT[j,i] = khatT.T @ qhatT per head, K=64 row-packing (2 heads
    concurrent in PE rows 0-63 / 64-127); evicted to bf16 `at` alternating
    DVE/ACT; out2T accumulated with M=64 col-packing
  - final projection aoT.T @ Wo with kt-outer loop (stationary reuse)
"""

import os
import sys

for _p in ("/opt/trn_rl_repo",):
    if os.path.isdir(_p) and _p not in sys.path:
        sys.path.insert(0, _p)

from contextlib import ExitStack

import ml_dtypes
import numpy as np

import concourse.bacc as bacc
import concourse.mybir as mybir
import concourse.tile as tile
from concourse import bass_utils

P = 128
F = 1024  # model dim
H = 16  # total heads
HD = 64  # head dim
G = 2  # head groups (tensor-parallel factor)
FG = F // G  # 512 features per core
PAIRS = FG // P  # 4 head-pairs per core
KT = F // P  # 8 contraction tiles for the projections
NCORES = 8
F32 = mybir.dt.float32
BF = mybir.dt.bfloat16
AF = mybir.ActivationFunctionType


def _mm(nc, out, lhsT, rhs, **kw):
    return nc.tensor.matmul(out, lhsT, rhs, **kw)


def build_core_program(nc, n=2048):
    NC = n // 512  # i-chunks
    NT = n // P  # n-tiles (= j-tiles)
    NTC = 512 // P  # n-tiles per i-chunk

    xt = nc.dram_tensor("xt", [P, NC, KT, 512], BF, kind="ExternalInput").ap()
    wq = nc.dram_tensor("wq", [P, PAIRS, KT, P], BF, kind="ExternalInput").ap()
    wk = nc.dram_tensor("wk", [P, PAIRS, KT, P], BF, kind="ExternalInput").ap()
    wv = nc.dram_tensor("wv", [P, KT, FG], BF, kind="ExternalInput").ap()
    wo = nc.dram_tensor("wo", [P, PAIRS, F], BF, kind="ExternalInput").ap()
    bqd = nc.dram_tensor("bq", [P, PAIRS], F32, kind="ExternalInput").ap()
    bkd = nc.dram_tensor("bk", [P, PAIRS], F32, kind="ExternalInput").ap()
    bvd = nc.dram_tensor("bv", [FG], BF, kind="ExternalInput").ap()
    # cmsq[a, p] = (n ** sigmoid(m))**2 for local head 2p+a
    cmsq = nc.dram_tensor("cmsq", [2, PAIRS], F32, kind="ExternalInput").ap()
    cind = nc.dram_tensor("cind", [2, P], BF, kind="ExternalInput").ap()
    cblk = nc.dram_tensor("cblk", [P, 2], BF, kind="ExternalInput").ap()
    cones = nc.dram_tensor("cones", [1, P], BF, kind="ExternalInput").ap()
    out = nc.dram_tensor("out", [n, F], BF, kind="ExternalOutput").ap()

    with tile.TileContext(nc) as tc, ExitStack() as ctx:
        const = ctx.enter_context(tc.tile_pool(name="const", bufs=1))
        persist = ctx.enter_context(tc.tile_pool(name="persist", bufs=1))
        ps = ctx.enter_context(tc.tile_pool(name="ps", bufs=1, space="PSUM"))
        work = ctx.enter_context(tc.tile_pool(name="work", bufs=1))

        # --- constants (issued on the ACT DGE queue so the tiny transfers
        # don't delay xall[0]/wv on the sync queue) ----------------------
        ones_blk = const.tile([P, 2], BF)  # block col-sums for head-pair norms
        nc.scalar.dma_start(ones_blk[:], cblk)
        ind = const.tile([2, P], BF)  # partition-broadcast indicator
        nc.scalar.dma_start(ind[:], cind)
        ones_row = const.tile([1, P], BF)  # bias outer-product row
        nc.scalar.dma_start(ones_row[:], cones)
        zcol = const.tile([P, 1], F32)  # explicit zero bias for ACT
        nc.any.memset(zcol[:], 0.0)

        bq_sb = const.tile([P, PAIRS], F32)
        nc.scalar.dma_start(bq_sb[:], bqd)
        bk_sb = const.tile([P, PAIRS], F32)
        nc.scalar.dma_start(bk_sb[:], bkd)
        bv_sb = const.tile([1, FG], BF)
        nc.scalar.dma_start(bv_sb[:], bvd[None, :])
        cm_sb = const.tile([2, PAIRS], F32)
        nc.scalar.dma_start(cm_sb[:], cmsq)

        # --- persistent activations -------------------------------------
        # DMA order matters: v-proj needs xall[0] + wv first; wq/wk next
        # (qk-proj starts ~40us in); wo last (needed only at the end).
        xall = persist.tile([P, NC, KT, 512], BF)
        wv_sb = persist.tile([P, KT, FG], BF)
        qT = persist.tile([P, PAIRS, n], BF)  # (x Wq + bq)^T, 2 heads/tile
        kT = persist.tile([P, PAIRS, n], BF)
        v = persist.tile([P, NT, FG], BF)  # x Wv + bv, natural layout
        aoT = persist.tile([P, PAIRS, n], BF)  # attn-out^T
        wq_sb = persist.tile([P, PAIRS, KT, P], BF)
        wk_sb = persist.tile([P, PAIRS, KT, P], BF)
        wo_sb = persist.tile([P, PAIRS, F], BF)
        nc.sync.dma_start(xall[:, 0], xt[:, 0])
        nc.scalar.dma_start(wv_sb[:], wv)  # concurrent with xall[0]
        for ic in range(1, NC):
            nc.sync.dma_start(xall[:, ic], xt[:, ic])
        nc.scalar.dma_start(wq_sb[:], wq)
        nc.scalar.dma_start(wk_sb[:], wk)
        nc.scalar.dma_start(wo_sb[:], wo)

        # ================= phase 1: v projection =========================
        for ic in range(NC):
            for jt in range(NTC):
                nt_idx = ic * NTC + jt
                jsl = slice(jt * P, (jt + 1) * P)
                pt = ps.tile([P, FG], F32, tag=f"big{nt_idx % 3}", bufs=1,
                             name=f"vacc{nt_idx % 3}")
                for k in range(KT):
                    _mm(nc, pt, xall[:, ic, k, jsl], wv_sb[:, k, :],
                        start=(k == 0), stop=False)
                # + 1s^T bv outer product adds the bias to every row
                _mm(nc, pt, ones_row, bv_sb, start=False, stop=True)
                nc.scalar.activation(v[:, nt_idx, :], pt, AF.Identity,
                                     bias=zcol[:])

        # ========== phase 2: q/k projections + all norms =================
        # Stationary-reuse: the second MM of each (ic0, ic1) pair shares the
        # weight tile with the first; ldweights=True lets it stream at the
        # full PE rate (no weight reload; probe-verified).  Requires the
        # pair to stay adjacent in the PE stream, so the norm chains (whose
        # matmuls become ready at unpredictable times) are emitted AFTER all
        # projections.  A scheduler reorder would corrupt the weights and
        # blow up the rel-err check.
        for ft in range(PAIRS):
            for wsb, bsb, dstT in ((wq_sb, bq_sb, qT), (wk_sb, bk_sb, kT)):
                for ich in range(NC // 2):
                    ics = (2 * ich, 2 * ich + 1)
                    pts = [ps.tile([P, 512], F32, tag=f"big{i}", bufs=1,
                                   name=f"qkacc{i}")
                           for i in range(2)]
                    for k in range(KT):
                        for i, ic in enumerate(ics):
                            _mm(nc, pts[i], wsb[:, ft, k, :],
                                xall[:, ic, k, :],
                                start=(k == 0), stop=(k == KT - 1),
                                skip_group_check=True)
                    for i, ic in enumerate(ics):
                        isl = slice(ic * 512, (ic + 1) * 512)
                        nc.scalar.activation(dstT[:, ft, isl], pts[i],
                                             AF.Identity,
                                             bias=bsb[:, ft:ft + 1])
        for ft in range(PAIRS):
            for dstT, scale_ap in ((qT, cm_sb[:, ft:ft + 1]), (kT, None)):
                # --- norm chain: 1/(||t|| * s) as [2, n] bf16 row ---------
                sq = work.tile([P, n], BF, tag="sq", bufs=2)
                nc.scalar.activation(sq[:], dstT[:, ft, :], AF.Square,
                                     bias=zcol[:])
                rowr = work.tile([2, n], BF, tag="rowr", bufs=2)
                for ch in range(NC):
                    csl = slice(ch * 512, (ch + 1) * 512)
                    nps = ps.tile([2, 512], F32, tag="av0", bufs=1, name="nps")
                    _mm(nc, nps, ones_blk, sq[:, csl], start=True, stop=True)
                    if scale_ap is not None:
                        nc.scalar.activation(rowr[:, csl], nps,
                                             AF.Abs_reciprocal_sqrt,
                                             bias=zcol[:2], scale=scale_ap)
                    else:
                        nc.scalar.activation(rowr[:, csl], nps,
                                             AF.Abs_reciprocal_sqrt,
                                             bias=zcol[:2])
                # broadcast row across partitions and apply in place
                for ch in range(NC):
                    csl = slice(ch * 512, (ch + 1) * 512)
                    bps = ps.tile([P, 512], F32, tag="av1", bufs=1, name="bps")
                    _mm(nc, bps, ind, rowr[:, csl], start=True, stop=True)
                    nc.vector.tensor_tensor(dstT[:, ft, csl],
                                            dstT[:, ft, csl],
                                            bps, mybir.AluOpType.mult)

        # ========== phase 3: cosine attention + output projection ========
        # Software-pipelined j-loop (lag 2): the PE queue is strict FIFO, so
        # av(j) — which waits on the cross-engine eviction of at(j) — must
        # sit BEHIND sim(j+1)/sim(j+2) in the queue or it head-of-line
        # blocks ready sims and the HAM clock re-throttles.
        # i-chunk-outer / pair-inner ordering so each chunk's output
        # projection (and its out DMA) interleaves with the next chunk's
        # attention instead of draining in a tail phase.
        LAG = 2
        for ic in range(NC):
            isl = slice(ic * 512, (ic + 1) * 512)
            for pr in range(PAIRS):
                avp = ps.tile([P, 512], F32, tag=f"av{pr % 2}", bufs=1,
                              name=f"av{pr % 2}")
                ats = {}

                def emit_sim(j):
                    jsl = slice(j * P, (j + 1) * P)
                    sp2 = ps.tile([P, 1024], F32, tag=f"big{j % 3}", bufs=1,
                                  name=f"sp{j % 3}")
                    for po in (0, HD):  # head 2pr (rows 0-63), 2pr+1
                        _mm(nc, sp2[:, 8 * po:8 * po + 512],
                            kT[po:po + HD, pr, jsl],
                            qT[po:po + HD, pr, isl],
                            start=True, stop=True, tile_position=(po, 0))
                    at = work.tile([P, 1024], BF, tag="at", bufs=6, name="at")
                    # whole-tile eviction on alternating engines: one sem for
                    # both av halves (keeps the av pair concurrent) and less
                    # per-op overhead; the lag-2 pipeline covers the latency
                    if j % 2 == 0:
                        nc.vector.tensor_copy(at[:], sp2)
                    else:
                        nc.scalar.copy(at[:], sp2)
                    ats[j] = at

                for j in range(LAG):
                    emit_sim(j)
                for j in range(NT):
                    if j + LAG < NT:
                        emit_sim(j + LAG)
                    at = ats.pop(j)
                    for po in (0, HD):
                        _mm(nc, avp[po:po + HD, :],
                            v[:, j, pr * P + po:pr * P + po + HD],
                            at[:, 8 * po:8 * po + 512],
                            start=(j == 0), stop=(j == NT - 1),
                            tile_position=(0, po), skip_group_check=True)
                if pr % 2 == 0:
                    nc.vector.tensor_copy(aoT[:, pr, isl], avp)
                else:
                    nc.scalar.copy(aoT[:, pr, isl], avp)

            # --- output projection for this i-chunk ----------------------
            # kt-outer: the aoT stationary is reused by the second fc MM
            # (ldweights=True), both fc accumulators live simultaneously
            for jt in range(NTC):
                nt = ic * NTC + jt
                ntsl = slice(nt * P, (nt + 1) * P)
                ost = work.tile([P, F], BF, tag="ost", bufs=2)
                pt2s = [ps.tile([P, 512], F32, tag=f"av{fc}", bufs=1,
                                name=f"av{fc}")
                        for fc in range(2)]
                for kt in range(PAIRS):
                    for fc in range(2):
                        fsl = slice(fc * 512, (fc + 1) * 512)
                        _mm(nc, pt2s[fc], aoT[:, kt, ntsl],
                            wo_sb[:, kt, fsl],
                            start=(kt == 0), stop=(kt == PAIRS - 1),
                            skip_group_check=True)
                for fc in range(2):
                    fsl = slice(fc * 512, (fc + 1) * 512)
                    if fc % 2 == 0:
                        nc.vector.tensor_copy(ost[:, fsl], pt2s[fc])
                    else:
                        nc.scalar.copy(ost[:, fsl], pt2s[fc])
                nc.sync.dma_start(out[ntsl, :], ost[:])
    return nc


_CACHE = {}


def _mark_ldw_reuse(nc):
    """In the final scheduled IR, mark every matmul whose stationary operand
    is identical to the immediately preceding matmul's with ldweights=True
    (skip the redundant weight reload).  Done post-compile so the schedule
    is already fixed — safe by construction."""
    n_mm = 0
    n_marked = 0
    for blk in nc.m.functions[0].blocks:
        prev_w = None
        for inst in blk.instructions:
            if not isinstance(inst, mybir.InstMatmult):
                continue
            n_mm += 1
            w = inst.ins[1]  # stationary PhysicalAccessPattern
            key = (str(w.memref), w.offset, str(w.ap), w.dtype,
                   str(inst.tile_position), str(inst.perf_mode))
            if key == prev_w and not inst.is_transpose:
                inst.ldweights = True
                n_marked += 1
            prev_w = key
    assert n_mm > 0, "ldw marking found no matmuls — wrong block traversal"
    return n_mm, n_marked


def get_nc(n=2048):
    if n not in _CACHE:
        nc = bacc.Bacc("TRN2", target_bir_lowering=False, debug=False,
                       num_devices=NCORES)
        build_core_program(nc, n)
        nc.compile()
        _mark_ldw_reuse(nc)
        _CACHE[n] = nc
    return _CACHE[n]


def _bf(a):
    return np.ascontiguousarray(a).astype(ml_dtypes.bfloat16)


def _warr(W, sl):
    return _bf(
        np.asarray(W, np.float32)[:, sl].reshape(KT, P, FG).transpose(1, 0, 2))


def _warr_ft(W, sl):
    return _bf(
        np.asarray(W, np.float32)[:, sl].reshape(KT, P, PAIRS, P)
        .transpose(1, 2, 0, 3))


_IND = np.zeros((2, P), np.float32)
_IND[0, :HD] = 1.0
_IND[1, HD:] = 1.0
_BLK = np.zeros((P, 2), np.float32)
_BLK[:HD, 0] = 1.0
_BLK[HD:, 1] = 1.0
_ONES = np.ones((1, P), np.float32)


def make_in_maps(x, Wq, bq, Wk, bk, Wv, bv, Wo, bo, m):
    n = x.shape[1]
    sig = 1.0 / (1.0 + np.exp(-np.asarray(m, np.float64)))
    scale = np.float64(n) ** sig  # [16] per-head n^sigmoid(m)
    NCc = n // 512
    # xt is shared by the two cores of each batch; weight transforms are
    # shared by the four cores of each head-group — build each variant once
    xts = [
        _bf(np.asarray(x[bi], np.float32)
            .reshape(NCc, 512, KT, P).transpose(3, 0, 2, 1))
        for bi in range(x.shape[0])
    ]
    gmaps = []
    for g in range(G):
        sl = slice(g * FG, (g + 1) * FG)
        hsc = scale[g * (H // G):(g + 1) * (H // G)]  # 8 local heads
        cm = (hsc ** 2).reshape(PAIRS, 2).T  # [2, PAIRS]
        gmaps.append({
            "wq": _warr_ft(Wq, sl), "wk": _warr_ft(Wk, sl), "wv": _warr(Wv, sl),
            "wo": _bf(
                np.asarray(Wo, np.float32)[sl].reshape(PAIRS, P, F)
                .transpose(1, 0, 2)),
            "bq": np.ascontiguousarray(np.asarray(bq, np.float32)[sl].reshape(PAIRS, P).T),
            "bk": np.ascontiguousarray(np.asarray(bk, np.float32)[sl].reshape(PAIRS, P).T),
            "bv": _bf(np.asarray(bv, np.float32)[sl]),
            "cmsq": np.ascontiguousarray(cm.astype(np.float32)),
            "cind": _bf(_IND),
            "cblk": _bf(_BLK),
            "cones": _bf(_ONES),
        })
    return [{"xt": xts[c // 2], **gmaps[c % 2]} for c in range(NCORES)]


def kernel(x, Wq, bq, Wk, bk, Wv, bv, Wo, bo, m, _trace=False):
    x = np.asarray(x, np.float32)
    b, n, f = x.shape
    nc = get_nc(n)
    in_maps = make_in_maps(x, Wq, bq, Wk, bk, Wv, bv, Wo, bo, m)
    res = bass_utils.run_bass_kernel_spmd(nc, in_maps,
                                          core_ids=list(range(NCORES)),
                                          trace=_trace)
    outs = [r["out"] for r in res.results]
    y = np.empty((b, n, f), np.float32)
    for bi in range(b):
        y[bi] = (outs[2 * bi].astype(np.float32)
                 + outs[2 * bi + 1].astype(np.float32))
    y += np.asarray(bo, np.float32).reshape(1, 1, f)
    if _trace:
        kernel._last_results = res
    return y


if __name__ == "__main__":
    # build-only smoke test (no device)
    nc = bacc.Bacc("TRN2", target_bir_lowering=False, debug=False,
                   num_devices=NCORES)
    build_core_program(nc, n=int(sys.argv[1]) if len(sys.argv) > 1 else 2048)
    print("build OK")


# revision 29
# speedup vs baseline: 1.0123x; 1.0123x over previous
"""Multi-head cosine self-attention on 8 Trainium2 NeuronCores (Bass/Tile).

Problem: y = MHA(x) with L2-normalized q/k (cosine attention) and per-head
scaling sim / n**sigmoid(m);  x: [4, 2048, 1024], 16 heads of dim 64.

Sharding: core c handles batch c//2 and head-group c%2 (8 heads = 512 of the
1024 q/k/v features).  Each core computes its partial output
(attn_out_part @ Wo[rows]); the host sums the two partials per batch and adds
bo.  No collectives.

Per-core pipeline (bf16 datapath, fp32 PSUM accumulation):
  - xT blocks stream to SBUF bf16; v = x Wv + bv (x-tile stationary)
  - qT/kT = W.T @ xT, k-outer loop with 2-chunk PSUM accumulators so each
    stationary weight tile is reused across i-chunks
  - row norms: ones-block matmul -> [2, n] PSUM; 1/(||q||*n^sig) computed in
    ONE Abs_reciprocal_sqrt activation (scale=(n^sig)^2); broadcast to 128
    partitions via a K=2 indicator matmul and applied in-place (DVE)
  - all norms for all 4 head-pairs are emitted before attention, so the PE
    stream stays dense and the HAM clock stays at 2.4 GHz
  - simT[j,i] = khatT.T @ qhatT per head, K=64 row-packing (2 heads
    concurrent in PE rows 0-63 / 64-127); evicted to bf16 `at` alternating
    DVE/ACT; out2T accumulated with M=64 col-packing
  - final projection aoT.T @ Wo with kt-outer loop (stationary reuse)
"""

import os
import sys

for _p in ("/opt/trn_rl_repo",):
    if os.path.isdir(_p) and _p not in sys.path:
        sys.path.insert(0, _p)

from contextlib import ExitStack

import ml_dtypes
import numpy as np

import concourse.bacc as bacc
import concourse.mybir as mybir
import concourse.tile as tile
from concourse import bass_utils

P = 128
F = 1024  # model dim
H = 16  # total heads
HD = 64  # head dim
G = 2  # head groups (tensor-parallel factor)
FG = F // G  # 512 features per core
PAIRS = FG // P  # 4 head-pairs per core
KT = F // P  # 8 contraction tiles for the projections
NCORES = 8
F32 = mybir.dt.float32
BF = mybir.dt.bfloat16
AF = mybir.ActivationFunctionType


def _mm(nc, out, lhsT, rhs, **kw):
    return nc.tensor.matmul(out, lhsT, rhs, **kw)


def build_core_program(nc, n=2048):
    NC = n // 512  # i-chunks
    NT = n // P  # n-tiles (= j-tiles)
    NTC = 512 // P  # n-tiles per i-chunk

    xt = nc.dram_tensor("xt", [P, NC, KT, 512], BF, kind="ExternalInput").ap()
    wq = nc.dram_tensor("wq", [P, PAIRS, KT, P], BF, kind="ExternalInput").ap()
    wk = nc.dram_tensor("wk", [P, PAIRS, KT, P], BF, kind="ExternalInput").ap()
    wv = nc.dram_tensor("wv", [P, KT, FG], BF, kind="ExternalInput").ap()
    wo = nc.dram_tensor("wo", [P, PAIRS, F], BF, kind="ExternalInput").ap()
    bqd = nc.dram_tensor("bq", [P, PAIRS], F32, kind="ExternalInput").ap()
    bkd = nc.dram_tensor("bk", [P, PAIRS], F32, kind="ExternalInput").ap()
    bvd = nc.dram_tensor("bv", [FG], BF, kind="ExternalInput").ap()
    # cmsq[a, p] = (n ** sigmoid(m))**2 for local head 2p+a
    cmsq = nc.dram_tensor("cmsq", [2, PAIRS], F32, kind="ExternalInput").ap()
    cind = nc.dram_tensor("cind", [2, P], BF, kind="ExternalInput").ap()
    cblk = nc.dram_tensor("cblk", [P, 2], BF, kind="ExternalInput").ap()
    cones = nc.dram_tensor("cones", [1, P], BF, kind="ExternalInput").ap()
    out = nc.dram_tensor("out", [n, F], BF, kind="ExternalOutput").ap()

    with tile.TileContext(nc) as tc, ExitStack() as ctx:
        const = ctx.enter_context(tc.tile_pool(name="const", bufs=1))
        persist = ctx.enter_context(tc.tile_pool(name="persist", bufs=1))
        ps = ctx.enter_context(tc.tile_pool(name="ps", bufs=1, space="PSUM"))
        work = ctx.enter_context(tc.tile_pool(name="work", bufs=1))

        # --- constants (issued on the ACT DGE queue so the tiny transfers
        # don't delay xall[0]/wv on the sync queue) ----------------------
        ones_blk = const.tile([P, 2], BF)  # block col-sums for head-pair norms
        nc.scalar.dma_start(ones_blk[:], cblk)
        ind = const.tile([2, P], BF)  # partition-broadcast indicator
        nc.scalar.dma_start(ind[:], cind)
        ones_row = const.tile([1, P], BF)  # bias outer-product row
        nc.scalar.dma_start(ones_row[:], cones)
        zcol = const.tile([P, 1], F32)  # explicit zero bias for ACT
        nc.any.memset(zcol[:], 0.0)

        bq_sb = const.tile([P, PAIRS], F32)
        nc.scalar.dma_start(bq_sb[:], bqd)
        bk_sb = const.tile([P, PAIRS], F32)
        nc.scalar.dma_start(bk_sb[:], bkd)
        bv_sb = const.tile([1, FG], BF)
        nc.scalar.dma_start(bv_sb[:], bvd[None, :])
        cm_sb = const.tile([2, PAIRS], F32)
        nc.scalar.dma_start(cm_sb[:], cmsq)

        # --- persistent activations -------------------------------------
        # DMA order matters: v-proj needs xall[0] + wv first; wq/wk next
        # (qk-proj starts ~40us in); wo last (needed only at the end).
        xall = persist.tile([P, NC, KT, 512], BF)
        wv_sb = persist.tile([P, KT, FG], BF)
        qT = persist.tile([P, PAIRS, n], BF)  # (x Wq + bq)^T, 2 heads/tile
        kT = persist.tile([P, PAIRS, n], BF)
        v = persist.tile([P, NT, FG], BF)  # x Wv + bv, natural layout
        aoT = persist.tile([P, PAIRS, n], BF)  # attn-out^T
        wq_sb = persist.tile([P, PAIRS, KT, P], BF)
        wk_sb = persist.tile([P, PAIRS, KT, P], BF)
        wo_sb = persist.tile([P, PAIRS, F], BF)
        # finer granularity on the startup-critical pieces: v-proj(ic0,jt0)
        # needs only xall[:,0,:,0:128] + the first wv k-blocks
        nc.scalar.dma_start(wv_sb[:, :2], wv[:, :2])
        for jt in range(NTC):
            jsl = slice(jt * P, (jt + 1) * P)
            nc.sync.dma_start(xall[:, 0, :, jsl], xt[:, 0, :, jsl])
        nc.scalar.dma_start(wv_sb[:, 2:], wv[:, 2:])
        for ic in range(1, NC):
            nc.sync.dma_start(xall[:, ic], xt[:, ic])
        nc.scalar.dma_start(wq_sb[:], wq)
        nc.scalar.dma_start(wk_sb[:], wk)
        nc.scalar.dma_start(wo_sb[:], wo)

        # ================= phase 1: v projection =========================
        for ic in range(NC):
            for jt in range(NTC):
                nt_idx = ic * NTC + jt
                jsl = slice(jt * P, (jt + 1) * P)
                pt = ps.tile([P, FG], F32, tag=f"big{nt_idx % 3}", bufs=1,
                             name=f"vacc{nt_idx % 3}")
                for k in range(KT):
                    _mm(nc, pt, xall[:, ic, k, jsl], wv_sb[:, k, :],
                        start=(k == 0), stop=False)
                # + 1s^T bv outer product adds the bias to every row
                _mm(nc, pt, ones_row, bv_sb, start=False, stop=True)
                nc.scalar.activation(v[:, nt_idx, :], pt, AF.Identity,
                                     bias=zcol[:])

        # ========== phase 2: q/k projections + all norms =================
        # Norm chains interleave with the projections per (ft, q/k) so the
        # ACT-bound chain latency hides under the projection MM stream and
        # the PE never idles long enough for the HAM clock to re-throttle.
        for ft in range(PAIRS):
            for wsb, bsb, dstT, scale_ap in (
                    (wq_sb, bq_sb, qT, cm_sb[:, ft:ft + 1]),
                    (wk_sb, bk_sb, kT, None)):
                for ich in range(NC // 2):
                    ics = (2 * ich, 2 * ich + 1)
                    pts = [ps.tile([P, 512], F32, tag=f"big{i}", bufs=1,
                                   name=f"qkacc{i}")
                           for i in range(2)]
                    for k in range(KT):
                        for i, ic in enumerate(ics):
                            _mm(nc, pts[i], wsb[:, ft, k, :],
                                xall[:, ic, k, :],
                                start=(k == 0), stop=(k == KT - 1),
                                skip_group_check=True)
                    for i, ic in enumerate(ics):
                        isl = slice(ic * 512, (ic + 1) * 512)
                        nc.scalar.activation(dstT[:, ft, isl], pts[i],
                                             AF.Identity,
                                             bias=bsb[:, ft:ft + 1])
                # --- norm chain: 1/(||t|| * s) as [2, n] bf16 row ---------
                # Square on DVE (tensor_tensor mult) to keep ACT free for
                # the rsqrt + eviction work
                sq = work.tile([P, n], BF, tag="sq", bufs=2)
                nc.vector.tensor_tensor(sq[:], dstT[:, ft, :], dstT[:, ft, :],
                                        mybir.AluOpType.mult)
                rowr = work.tile([2, n], BF, tag="rowr", bufs=2)
                for ch in range(NC):
                    csl = slice(ch * 512, (ch + 1) * 512)
                    nps = ps.tile([2, 512], F32, tag="av0", bufs=1, name="nps")
                    _mm(nc, nps, ones_blk, sq[:, csl], start=True, stop=True)
                    if scale_ap is not None:
                        nc.scalar.activation(rowr[:, csl], nps,
                                             AF.Abs_reciprocal_sqrt,
                                             bias=zcol[:2], scale=scale_ap)
                    else:
                        nc.scalar.activation(rowr[:, csl], nps,
                                             AF.Abs_reciprocal_sqrt,
                                             bias=zcol[:2])
                # broadcast row across partitions and apply in place
                for ch in range(NC):
                    csl = slice(ch * 512, (ch + 1) * 512)
                    bps = ps.tile([P, 512], F32, tag="av1", bufs=1, name="bps")
                    _mm(nc, bps, ind, rowr[:, csl], start=True, stop=True)
                    nc.vector.tensor_tensor(dstT[:, ft, csl],
                                            dstT[:, ft, csl],
                                            bps, mybir.AluOpType.mult)

        # ========== phase 3: cosine attention + output projection ========
        # Software-pipelined j-loop (lag 2): the PE queue is strict FIFO, so
        # av(j) — which waits on the cross-engine eviction of at(j) — must
        # sit BEHIND sim(j+1)/sim(j+2) in the queue or it head-of-line
        # blocks ready sims and the HAM clock re-throttles.
        # i-chunk-outer / pair-inner ordering so each chunk's output
        # projection (and its out DMA) interleaves with the next chunk's
        # attention instead of draining in a tail phase.
        LAG = 2
        for ic in range(NC):
            isl = slice(ic * 512, (ic + 1) * 512)
            for pr in range(PAIRS):
                avp = ps.tile([P, 512], F32, tag=f"av{pr % 2}", bufs=1,
                              name=f"av{pr % 2}")
                ats = {}

                def emit_sim(j):
                    jsl = slice(j * P, (j + 1) * P)
                    sp2 = ps.tile([P, 1024], F32, tag=f"big{j % 3}", bufs=1,
                                  name=f"sp{j % 3}")
                    for po in (0, HD):  # head 2pr (rows 0-63), 2pr+1
                        _mm(nc, sp2[:, 8 * po:8 * po + 512],
                            kT[po:po + HD, pr, jsl],
                            qT[po:po + HD, pr, isl],
                            start=True, stop=True, tile_position=(po, 0))
                    at = work.tile([P, 1024], BF, tag="at", bufs=6, name="at")
                    # whole-tile eviction on alternating engines: one sem for
                    # both av halves (keeps the av pair concurrent) and less
                    # per-op overhead; the lag-2 pipeline covers the latency
                    if j % 2 == 0:
                        nc.vector.tensor_copy(at[:], sp2)
                    else:
                        nc.scalar.copy(at[:], sp2)
                    ats[j] = at

                for j in range(LAG):
                    emit_sim(j)
                for j in range(NT):
                    if j + LAG < NT:
                        emit_sim(j + LAG)
                    at = ats.pop(j)
                    for po in (0, HD):
                        _mm(nc, avp[po:po + HD, :],
                            v[:, j, pr * P + po:pr * P + po + HD],
                            at[:, 8 * po:8 * po + 512],
                            start=(j == 0), stop=(j == NT - 1),
                            tile_position=(0, po), skip_group_check=True)
                if pr % 2 == 0:
                    nc.vector.tensor_copy(aoT[:, pr, isl], avp)
                else:
                    nc.scalar.copy(aoT[:, pr, isl], avp)

            # --- output projection for this i-chunk ----------------------
            # kt-outer: the aoT stationary is reused by the second fc MM
            # (ldweights=True), both fc accumulators live simultaneously
            for jt in range(NTC):
                nt = ic * NTC + jt
                ntsl = slice(nt * P, (nt + 1) * P)
                ost = work.tile([P, F], BF, tag="ost", bufs=2)
                pt2s = [ps.tile([P, 512], F32, tag=f"av{fc}", bufs=1,
                                name=f"av{fc}")
                        for fc in range(2)]
                for kt in range(PAIRS):
                    for fc in range(2):
                        fsl = slice(fc * 512, (fc + 1) * 512)
                        _mm(nc, pt2s[fc], aoT[:, kt, ntsl],
                            wo_sb[:, kt, fsl],
                            start=(kt == 0), stop=(kt == PAIRS - 1),
                            skip_group_check=True)
                for fc in range(2):
                    fsl = slice(fc * 512, (fc + 1) * 512)
                    if fc % 2 == 0:
                        nc.vector.tensor_copy(ost[:, fsl], pt2s[fc])
                    else:
                        nc.scalar.copy(ost[:, fsl], pt2s[fc])
                nc.sync.dma_start(out[ntsl, :], ost[:])
    return nc


_CACHE = {}


def _mark_ldw_reuse(nc):
    """In the final scheduled IR, mark every matmul whose stationary operand
    is identical to the immediately preceding matmul's with ldweights=True
    (skip the redundant weight reload).  Done post-compile so the schedule
    is already fixed — safe by construction."""
    n_mm = 0
    n_marked = 0
    for blk in nc.m.functions[0].blocks:
        prev_w = None
        for inst in blk.instructions:
            if not isinstance(inst, mybir.InstMatmult):
                continue
            n_mm += 1
            w = inst.ins[1]  # stationary PhysicalAccessPattern
            key = (str(w.memref), w.offset, str(w.ap), w.dtype,
                   str(inst.tile_position), str(inst.perf_mode))
            if key == prev_w and not inst.is_transpose:
                inst.ldweights = True
                n_marked += 1
            prev_w = key
    assert n_mm > 0, "ldw marking found no matmuls — wrong block traversal"
    return n_mm, n_marked


def get_nc(n=2048):
    if n not in _CACHE:
        nc = bacc.Bacc("TRN2", target_bir_lowering=False, debug=False,
                       num_devices=NCORES)
        build_core_program(nc, n)
        nc.compile()
        _mark_ldw_reuse(nc)
        _CACHE[n] = nc
    return _CACHE[n]


def _bf(a):
    return np.ascontiguousarray(a).astype(ml_dtypes.bfloat16)


def _warr(W, sl):
    return _bf(
        np.asarray(W, np.float32)[:, sl].reshape(KT, P, FG).transpose(1, 0, 2))


def _warr_ft(W, sl):
    return _bf(
        np.asarray(W, np.float32)[:, sl].reshape(KT, P, PAIRS, P)
        .transpose(1, 2, 0, 3))


_IND = np.zeros((2, P), np.float32)
_IND[0, :HD] = 1.0
_IND[1, HD:] = 1.0
_BLK = np.zeros((P, 2), np.float32)
_BLK[:HD, 0] = 1.0
_BLK[HD:, 1] = 1.0
_ONES = np.ones((1, P), np.float32)


def make_in_maps(x, Wq, bq, Wk, bk, Wv, bv, Wo, bo, m):
    n = x.shape[1]
    sig = 1.0 / (1.0 + np.exp(-np.asarray(m, np.float64)))
    scale = np.float64(n) ** sig  # [16] per-head n^sigmoid(m)
    NCc = n // 512
    # xt is shared by the two cores of each batch; weight transforms are
    # shared by the four cores of each head-group — build each variant once
    xts = [
        _bf(np.asarray(x[bi], np.float32)
            .reshape(NCc, 512, KT, P).transpose(3, 0, 2, 1))
        for bi in range(x.shape[0])
    ]
    gmaps = []
    for g in range(G):
        sl = slice(g * FG, (g + 1) * FG)
        hsc = scale[g * (H // G):(g + 1) * (H // G)]  # 8 local heads
        cm = (hsc ** 2).reshape(PAIRS, 2).T  # [2, PAIRS]
        gmaps.append({
            "wq": _warr_ft(Wq, sl), "wk": _warr_ft(Wk, sl), "wv": _warr(Wv, sl),
            "wo": _bf(
                np.asarray(Wo, np.float32)[sl].reshape(PAIRS, P, F)
                .transpose(1, 0, 2)),
            "bq": np.ascontiguousarray(np.asarray(bq, np.float32)[sl].reshape(PAIRS, P).T),
            "bk": np.ascontiguousarray(np.asarray(bk, np.float32)[sl].reshape(PAIRS, P).T),
            "bv": _bf(np.asarray(bv, np.float32)[sl]),
            "cmsq": np.ascontiguousarray(cm.astype(np.float32)),
            "cind": _bf(_IND),
            "cblk": _bf(_BLK),
            "cones": _bf(_ONES),
        })
    return [{"xt": xts[c // 2], **gmaps[c % 2]} for c in range(NCORES)]


def kernel(x, Wq, bq, Wk, bk, Wv, bv, Wo, bo, m, _trace=False):
    x = np.asarray(x, np.float32)
    b, n, f = x.shape
    nc = get_nc(n)
    in_maps = make_in_maps(x, Wq, bq, Wk, bk, Wv, bv, Wo, bo, m)
    res = bass_utils.run_bass_kernel_spmd(nc, in_maps,
                                          core_ids=list(range(NCORES)),
                                          trace=_trace)
    outs = [r["out"] for r in res.results]
    y = np.empty((b, n, f), np.float32)
    for bi in range(b):
        y[bi] = (outs[2 * bi].astype(np.float32)
                 + outs[2 * bi + 1].astype(np.float32))
    y += np.asarray(bo, np.float32).reshape(1, 1, f)
    if _trace:
        kernel._last_results = res
    return y


if __name__ == "__main__":
    # build-only smoke test (no device)
    nc = bacc.Bacc("TRN2", target_bir_lowering=False, debug=False,
                   num_devices=NCORES)
    build_core_program(nc, n=int(sys.argv[1]) if len(sys.argv) > 1 else 2048)
    print("build OK")
